# revision 11
# baseline (speedup 1.0000x reference)
"""Trainium2 Bass kernel for nn_CMEncoder (cross-attention + LayerNorm2d + MLP block).

Strategy (8 NeuronCores, sequence-parallel over the HW=4096 query tokens; each
core owns 512 queries, full 4096-token context):

  Host-side algebraic folds:
    - G = Wq^T @ Wk  so that  scores S = x^T G y.  The K projection and its
      PSUM evacuation disappear: on-chip we compute qg = G^T x once and
      contract score chunks directly against the DMA'd y (channel major).
      bq folds in exactly via qg += (Wk^T bq) (x) ones.
    - H = Wo @ Wv  so that  z = H (y P) = H U.  The V projection, its PSUM
      evacuation, and the Wo projection all collapse into one 256x256 panel
      applied to U = y_tm @ P (y also DMA'd token-major).
    - bk dropped (softmax shift invariance), bv/bo folded to bo', LayerNorm
      affine folded into W1/b1.
    - When bo' == 0 (true here), softmax's denominator cancels inside
      LayerNorm (per-token scale invariance): the column-sum matmuls and the
      whole 1/d chain are deleted; LN's eps is absorbed analytically with
      d ~= 4096*sqrt(e).
    - LN's per-token affine is zA = zs * bcast(A); the mean subtraction is
      folded into MLP1 as a rank-1 matmul (-w1s/C) (x) (A.*Sz).

  Device schedule: inputs arrive via few, large, need-ordered DMAs (each
  dma_start costs ~0.6us of queue issue time); PE warm-up matmuls run during
  the DMA window and dummy filler matmuls keep the PE clock at 2.4 GHz across
  the post-loop LayerNorm chain.  The attention loop (16 macro-chunks of 256
  context tokens) is software-pipelined: 4 score MMs -> one [128,1024] exp on
  ACT -> 4 U MMs, with y streaming in under the loop.  All matmuls bf16 with
  fp32 PSUM accumulation.
"""

import math
import numpy as np
import concourse.bacc as bacc
import concourse.mybir as mybir
import concourse.tile as tile
from concourse import bass_utils
from concourse.hw_specs import get_activation_tables

F32 = mybir.dt.float32
F32R = mybir.dt.float32r
BF16 = mybir.dt.bfloat16
AF = mybir.ActivationFunctionType
ALU = mybir.AluOpType

MMDT = BF16

C = 256          # channels
HW = 4096        # query tokens (64x64)
NCTX = 4096      # context tokens
HID = 512        # mlp hidden
NCORES = 8
QS = HW // NCORES    # 512 queries per core
NMAC = 16            # macro chunks of 256 context tokens
EPS = 1e-6
N_WARM = 14          # PE warm-up matmuls during the DMA window
WALL_W = 512 + 1024 + 1024 + 4


def _build_nc(has_bq=False, has_bo=False, vb=0.0):
    nc = bacc.Bacc("TRN2", target_bir_lowering=False)

    # --- DRAM I/O (batched: few large transfers; dma_start issue is ~0.6us) ---
    d_xg = nc.dram_tensor("xg", (128, 1536), MMDT, kind="ExternalInput")
    d_yc = nc.dram_tensor("y_cm", (128, 4 * 2048), MMDT, kind="ExternalInput")
    d_yt = nc.dram_tensor("y_tm", (128, 2 * 16 * C), MMDT, kind="ExternalInput")
    d_wall = nc.dram_tensor("wall", (128, WALL_W), MMDT, kind="ExternalInput")
    d_rw = nc.dram_tensor("rows_mm", (1, C + 2 * HID), F32R, kind="ExternalInput")
    d_sm = nc.dram_tensor("smalls", (128, 6), F32, kind="ExternalInput")
    d_xf = nc.dram_tensor("x_f32", (128, 1024), F32, kind="ExternalInput")
    d_out = nc.dram_tensor("out_sh", (C, QS), F32, kind="ExternalOutput")

    tabs = list(get_activation_tables(nc.m.arch).keys())
    LNEXP_SET = tabs.index("natural_log_exp_and_others")

    with tile.TileContext(nc) as tc:
        # Pre-load the exp+ln activation table once so the auto-inserted loads
        # don't ping-pong between sets mid-kernel.
        nc.scalar.add_instruction(mybir.InstLoadActFuncSet(
            name=nc.get_next_instruction_name(), ins=[], outs=[],
            act_func_set_id=LNEXP_SET))

        with (
            tc.tile_pool(name="sb", bufs=1) as sb,
            tc.tile_pool(name="pt_pool", bufs=3) as ptp,
            tc.tile_pool(name="ps1024", bufs=2, space="PSUM") as psL,
            tc.tile_pool(name="psacc", bufs=1, space="PSUM") as psA,
            tc.tile_pool(name="psw", bufs=(1 if has_bo else 2), space="PSUM") as psW,
        ):
            # ---------------- input DMAs, need-ordered ----------------
            xg = sb.tile([128, 1536], MMDT)
            yc = [sb.tile([128, 2, 1024], MMDT, name=f"yc{i}") for i in range(4)]
            yt = [sb.tile([128, 16, C], MMDT, name=f"yt{i}") for i in range(2)]
            wall = sb.tile([128, WALL_W], MMDT)
            rows = sb.tile([1, C + 2 * HID], F32R)
            smalls = sb.tile([128, 6], F32)
            xfb = sb.tile([128, 1024], F32)

            nc.sync.dma_start(xg, d_xg[:, :])
            nc.sync.dma_start(yc[0], d_yc[:, 0:2048])
            nc.sync.dma_start(yc[2], d_yc[:, 2 * 2048:3 * 2048])
            nc.sync.dma_start(wall, d_wall[:, :])
            nc.sync.dma_start(rows, d_rw[:, :])
            nc.sync.dma_start(smalls, d_sm[:, :])

            nc.gpsimd.dma_start(yt[0], d_yt[:, 0:16 * C])
            nc.gpsimd.dma_start(yc[1], d_yc[:, 2048:2 * 2048])
            nc.gpsimd.dma_start(yt[1], d_yt[:, 16 * C:32 * C])
            nc.gpsimd.dma_start(yc[3], d_yc[:, 3 * 2048:4 * 2048])
            nc.gpsimd.dma_start(xfb, d_xf[:, :])

            xmm = xg[:, 0:1024]
            g_t = xg[:, 1024:1536]
            h_t = wall[:, 0:512]
            w1_t = wall[:, 512:1536]
            w2_t = wall[:, 1536:2560]
            hsb = wall[:, 2560:2564]
            r_row = rows[0:1, 0:C]
            w1s_row = rows[0:1, C:C + HID]
            w1b_row = rows[0:1, C + HID:C + 2 * HID]
            b1p = smalls[:, 0:4]
            b2c = smalls[:, 4:6]
            xf = [xfb[:, 0:512], xfb[:, 512:1024]]

            # constants (memset only supports fp32; bf16/f32r come via copies)
            or32 = sb.tile([1, 128], F32)
            nc.vector.memset(or32, 1.0)
            ones_r1 = sb.tile([1, 128], F32R)
            nc.vector.tensor_copy(ones_r1, or32)
            oc32 = sb.tile([128, 1], F32)
            nc.vector.memset(oc32, 1.0)
            ones_c1 = sb.tile([128, 1], MMDT)
            nc.vector.tensor_copy(ones_c1, oc32)
            ws32 = sb.tile([128, 128], F32)
            nc.vector.memset(ws32, 0.015625)
            ws = sb.tile([128, 128], MMDT)
            nc.vector.tensor_copy(ws, ws32)

            epsb = sb.tile([1, 1], F32)
            lnCv = sb.tile([1, 1], F32)
            nc.vector.memset(lnCv, math.log(float(C)))
            if has_bo:
                # general: X = a^2 v1r + 2C a c2r + C^2(vb+eps)
                nc.vector.memset(epsb, float(C * C) * (vb + EPS))
            else:
                # eps absorbed with d ~= NCTX*sqrt(e): bias = C^2 * eps * d^2
                nc.vector.memset(epsb, float(C * C) * EPS * (NCTX * math.exp(0.5)) ** 2)

            def wsl(t, cc, cb, w=128):
                return t[:, cc * (t.shape[1] // 2) + cb * w:
                         cc * (t.shape[1] // 2) + (cb + 1) * w]

            def wslw(base, width, cc, cb, w=128):
                # slice within the packed `wall` blob
                off = base + cc * (width // 2) + cb * w
                return wall[:, off:off + w]

            # ---------------- PE warm-up during the DMA window ----------------
            for i in range(N_WARM):
                wps = psL.tile([128, 1024], F32, tag="sps", name=f"warm{i % 2}")
                nc.tensor.matmul(wps[:, 0:128], ws, ws, start=True, stop=True)

            # ---------------- qg = G^T x (+ (Wk^T bq) (x) ones) ----------------
            if has_bq:
                rbf = sb.tile([1, C], MMDT)
                nc.vector.tensor_copy(rbf, r_row)
                oq32 = sb.tile([1, QS], F32)
                nc.vector.memset(oq32, 1.0)
                ones_qb = sb.tile([1, QS], MMDT)
                nc.vector.tensor_copy(ones_qb, oq32)
            qg_ps = psL.tile([128, 1024], F32, tag="sps", name="qg_ps")
            for cb in range(2):
                for cc in range(2):
                    nc.tensor.matmul(qg_ps[:, cb * 512:(cb + 1) * 512],
                                     g_t[:, cc * 256 + cb * 128:cc * 256 + (cb + 1) * 128],
                                     xmm[:, cc * QS:(cc + 1) * QS],
                                     start=(cc == 0), stop=(cc == 1 and not has_bq))
                if has_bq:
                    nc.tensor.matmul(qg_ps[:, cb * 512:(cb + 1) * 512],
                                     rbf[0:1, cb * 128:(cb + 1) * 128],
                                     ones_qb, start=False, stop=True)
            qgs = sb.tile([128, 1024], MMDT)
            nc.scalar.copy(qgs, qg_ps)

            # ---------------- attention loop ----------------
            u_ps = [psA.tile([128, 512], F32, tag=f"u{cb}", name=f"u{cb}")
                    for cb in range(2)]
            if has_bo:
                csum = psA.tile([1, QS], F32, tag="csum", name="csum")

            def scores(m):
                sps = psL.tile([128, 1024], F32, tag="sps", name=f"sps{m % 2}")
                for h in range(2):
                    j = 2 * m + h
                    for cb in range(2):
                        nc.tensor.matmul(
                            sps[:, h * 512:(h + 1) * 512],
                            yc[j // 8][:, cb, (j % 8) * 128:(j % 8) * 128 + 128],
                            qgs[:, cb * 512:(cb + 1) * 512],
                            start=(cb == 0), stop=(cb == 1))
                pt = ptp.tile([128, 1024], MMDT, tag="pt", name=f"pt{m % 3}")
                nc.scalar.activation(pt, sps, AF.Exp, scale=1.0 / 16.0)
                return pt

            def accum(m, pt, csum_first=False):
                first, last = (m == 0), (m == NMAC - 1)
                if has_bo and csum_first:
                    for h in range(2):
                        nc.tensor.matmul(csum, ones_c1, pt[:, h * 512:(h + 1) * 512],
                                         start=(first and h == 0), stop=(last and h == 1))
                for h in range(2):
                    j = 2 * m + h
                    for cb in range(2):
                        nc.tensor.matmul(
                            u_ps[cb],
                            yt[j // 16][:, j % 16, cb * 128:(cb + 1) * 128],
                            pt[:, h * 512:(h + 1) * 512],
                            start=(first and h == 0), stop=(last and h == 1))
                if has_bo and not csum_first:
                    for h in range(2):
                        nc.tensor.matmul(csum, ones_c1, pt[:, h * 512:(h + 1) * 512],
                                         start=(first and h == 0), stop=(last and h == 1))

            prev = scores(0)
            for m in range(1, NMAC):
                cur = scores(m)
                accum(m - 1, prev)
                prev = cur
            accum(NMAC - 1, prev, csum_first=True)

            # dummy filler matmuls: keep the PE clock warm over engine handoffs
            def fill(n, where, wslot=None):
                for i in range(n):
                    if wslot is None:
                        t = psW.tile([128, 512], F32, tag="work", name=f"fw_{where}")
                        nc.tensor.matmul(t[:, 0:512], ws, ws32_mv, start=True, stop=True)
                    else:
                        nc.tensor.matmul(wslot[:, 0:512], ws, ws32_mv,
                                         start=True, stop=True)

            ws32_mv = sb.tile([128, 512], MMDT)
            # (built from ws columns; a cheap DVE broadcast-ish copy)
            nc.vector.tensor_copy(ws32_mv[:, 0:128], ws)
            nc.vector.tensor_copy(ws32_mv[:, 128:256], ws)
            nc.vector.tensor_copy(ws32_mv[:, 256:384], ws)
            nc.vector.tensor_copy(ws32_mv[:, 384:512], ws)

            # ---------------- 1/d chain (general path only) ----------------
            if has_bo:
                lncs = sb.tile([1, QS], F32)
                nc.scalar.activation(lncs, csum[0:1, :], AF.Ln)
                alpha = sb.tile([1, QS], F32)
                nc.scalar.activation(alpha, lncs, AF.Exp, scale=-1.0)

            # ---------------- U evac + z~ = H U + stats ----------------
            # D0 fillers run while ACT/DVE evacuate U
            fill(3, "d0")

            uq = sb.tile([128, 1024], MMDT)
            nc.scalar.copy(uq[:, 0:512], u_ps[0])
            nc.vector.tensor_copy(uq[:, 512:1024], u_ps[1])

            zt_ps = psL.tile([128, 1024], F32, tag="sps", name="zt_ps")
            for cb in range(2):
                for cc in range(2):
                    nc.tensor.matmul(zt_ps[:, cb * 512:(cb + 1) * 512],
                                     wslw(0, 512, cc, cb), uq[:, cc * 512:(cc + 1) * 512],
                                     start=(cc == 0), stop=(cc == 1))

            nstat = 2 if has_bo else 1
            stat_ps = psW.tile([2, QS], F32, tag="work", name="stat_ps")
            for cc in range(2):
                nc.tensor.matmul(stat_ps[0:nstat, :], hsb[:, cc * 2:cc * 2 + nstat],
                                 uq[:, cc * 512:(cc + 1) * 512],
                                 start=(cc == 0), stop=(cc == 1))
            stat_sb = sb.tile([2, QS], F32R)
            nc.vector.tensor_copy(stat_sb[0:nstat, :], stat_ps[0:nstat, :])
            sz_row = stat_sb[0:1, :]
            s2 = sb.tile([1, QS], F32)
            nc.scalar.square(s2, sz_row)

            # zs (DVE) whole; zsq (ACT) in halves so the Sq matmuls start early
            zs = sb.tile([128, 1024], MMDT)
            nc.vector.tensor_copy(zs, zt_ps)
            zsq = sb.tile([128, 1024], MMDT)
            sq_ps = psW.tile([2, QS], F32, tag="work", name="sq_ps")
            fill(4, "d1", u_ps[0])
            for cc in range(2):
                nc.scalar.square(zsq[:, cc * 512:(cc + 1) * 512],
                                 zt_ps[:, cc * 512:(cc + 1) * 512])
                nc.tensor.matmul(sq_ps[0:1, :], ones_c1, zsq[:, cc * 512:(cc + 1) * 512],
                                 start=(cc == 0), stop=(cc == 1))

            # v1r = C*Sq - Sz^2 ;  X = v1r (+eps bias)  or the general alpha form
            v1r = sb.tile([1, QS], F32)
            nc.vector.scalar_tensor_tensor(v1r, sq_ps[0:1, :], float(C), s2,
                                           op0=ALU.mult, op1=ALU.subtract)
            lnX = sb.tile([1, QS], F32)
            A_row = sb.tile([1, QS], F32R)
            fill(10, "d2", u_ps[0])
            if has_bo:
                t1 = sb.tile([1, QS], F32)
                nc.vector.tensor_mul(t1, alpha, v1r)
                t2 = sb.tile([1, QS], F32)
                nc.vector.scalar_tensor_tensor(t2, stat_sb[1:2, :], 2.0 * C, t1,
                                               op0=ALU.mult, op1=ALU.add)
                X = sb.tile([1, QS], F32)
                nc.vector.tensor_mul(X, alpha, t2)
                nc.scalar.activation(lnX, X, AF.Ln, bias=epsb)
                preA = sb.tile([1, QS], F32)
                nc.vector.scalar_tensor_tensor(preA, lnX, -0.5, lncs,
                                               op0=ALU.mult, op1=ALU.subtract)
                nc.scalar.activation(A_row, preA, AF.Exp, bias=lnCv)
                R_row = sb.tile([1, QS], F32R)
                nc.scalar.activation(R_row, lnX, AF.Exp, scale=-0.5, bias=lnCv)
            else:
                nc.scalar.activation(lnX, v1r, AF.Ln, bias=epsb)
                nc.scalar.activation(A_row, lnX, AF.Exp, scale=-0.5, bias=lnCv)

            mm1 = sb.tile([1, QS], F32R)
            nc.vector.tensor_mul(mm1, A_row, sz_row)

            ab_ps = psW.tile([128, QS], F32, tag="work", name="ab_ps")
            nc.tensor.matmul(ab_ps, ones_r1, A_row, start=True, stop=True)

            fill(5, "d3", u_ps[0])

            zA = sb.tile([128, 1024], MMDT)
            for cb in range(2):
                nc.vector.tensor_mul(zA[:, cb * 512:(cb + 1) * 512],
                                     zs[:, cb * 512:(cb + 1) * 512], ab_ps)

            # ---------------- MLP1 (+rank-1 mean subtraction) + gelu ----------------
            hs = [sb.tile([128, QS], MMDT, name=f"hs{i}") for i in range(4)]
            for hb in range(4):
                hps = psW.tile([128, QS], F32, tag="work", name=f"hps{hb % 2}")
                for cb in range(2):
                    nc.tensor.matmul(hps, wslw(512, 1024, cb, hb),
                                     zA[:, cb * 512:(cb + 1) * 512],
                                     start=(cb == 0), stop=False)
                nc.tensor.matmul(hps, w1s_row[0:1, hb * 128:(hb + 1) * 128], mm1,
                                 start=False, stop=(not has_bo))
                if has_bo:
                    nc.tensor.matmul(hps, w1b_row[0:1, hb * 128:(hb + 1) * 128], R_row,
                                     start=False, stop=True)
                nc.scalar.activation(hs[hb], hps, AF.Gelu, bias=b1p[:, hb:hb + 1])

            # ---------------- MLP2 + residual + out ----------------
            for cb in range(2):
                tps2 = psW.tile([128, QS], F32, tag="work", name=f"tps2{cb}")
                for hb in range(4):
                    nc.tensor.matmul(
                        tps2, w2_t[:, hb * 256 + cb * 128:hb * 256 + (cb + 1) * 128],
                        hs[hb], start=(hb == 0), stop=(hb == 3))
                ot = sb.tile([128, QS], F32, name=f"ot{cb}")
                nc.vector.scalar_tensor_tensor(ot, tps2, b2c[:, cb:cb + 1], xf[cb],
                                               op0=ALU.add, op1=ALU.add)
                nc.sync.dma_start(d_out[cb * 128:(cb + 1) * 128, :], ot)

    nc.compile()
    return nc


_NC = None
_NC_KEY = None


def _get_nc(has_bq=None, has_bo=None, vb=0.0):
    global _NC, _NC_KEY
    if has_bq is None:
        if _NC is not None:
            return _NC
        has_bq, has_bo = False, False
    key = (has_bq, has_bo, vb)
    if _NC is None or _NC_KEY != key:
        _NC = _build_nc(has_bq=has_bq, has_bo=has_bo, vb=vb)
        _NC_KEY = key
    return _NC


def _pack_rows(a, nchunk):
    """(nchunk*128, W) -> (128, nchunk*W) with row-chunks side by side."""
    w = a.shape[1]
    out = np.empty((128, nchunk * w), a.dtype)
    for i in range(nchunk):
        out[:, i * w:(i + 1) * w] = a[i * 128:(i + 1) * 128, :]
    return out


def prep_in_maps(x, y, Wq, bq, Wk, bk, Wv, bv, Wo, bo, ln_w, ln_b, W1, b1, W2, b2):
    f = lambda a: np.asarray(a, dtype=np.float32)
    x, y = f(x), f(y)
    Wq, bq, Wk, Wv, bv, Wo, bo = f(Wq), f(bq), f(Wk), f(Wv), f(bv), f(Wo), f(bo)
    ln_w, ln_b, W1, b1, W2, b2 = f(ln_w), f(ln_b), f(W1), f(b1), f(W2), f(b2)

    mmnp = mybir.dt.np(MMDT)
    g = lambda a: np.ascontiguousarray(a).astype(mmnp)

    x_cm = np.ascontiguousarray(x.reshape(C, HW))
    y_cm = np.ascontiguousarray(y.reshape(C, NCTX))

    # host-side algebraic folds (fp64)
    G = Wq.astype(np.float64).T @ Wk.astype(np.float64)      # S = x^T G y
    H = Wo.astype(np.float64) @ Wv.astype(np.float64)        # z~ = H U
    r_vec = Wk.astype(np.float64).T @ bq.astype(np.float64)  # bq fold
    bo_p = (Wo.astype(np.float64) @ bv.astype(np.float64) + bo).astype(np.float64)
    W1p = (W1.astype(np.float64) * ln_w[None, :].astype(np.float64))
    b1_p = (W1.astype(np.float64) @ ln_b.astype(np.float64) + b1).astype(np.float32)

    has_bq = bool(np.abs(r_vec).max() > 0)
    has_bo = bool(np.abs(bo_p).max() > 0)

    b_bar = float(bo_p.mean())
    vb = float((bo_p ** 2).mean() - b_bar ** 2)
    hs_vec = H.sum(axis=0)                      # Sz = hs^T U
    hb_vec = H.T @ bo_p - b_bar * hs_vec        # c2r = hb^T U
    w1s = W1p.sum(axis=1)                       # W1p @ ones
    w1b = W1p @ bo_p                            # W1p @ bo'

    w1s_row = (-w1s / C).astype(np.float32)
    w1b_row = (w1b - b_bar * w1s).astype(np.float32)

    # stats stationary: per c'-block 2 cols [hs | hb]
    hsb = np.zeros((128, 4), np.float64)
    for cc in range(2):
        hsb[:, cc * 2 + 0] = hs_vec[cc * 128:(cc + 1) * 128]
        hsb[:, cc * 2 + 1] = hb_vec[cc * 128:(cc + 1) * 128]

    # y_cm pieces: [128, 4 pieces, 2 cb, 1024 tok]
    y_cm_pk = np.empty((128, 4 * 2048), np.float32)
    for p in range(4):
        for cb in range(2):
            y_cm_pk[:, p * 2048 + cb * 1024:p * 2048 + (cb + 1) * 1024] = \
                y_cm[cb * 128:(cb + 1) * 128, p * 1024:(p + 1) * 1024]
    # y_tm: token-major [128, 32 chunks, 256 ch]
    y_tm = np.ascontiguousarray(
        y_cm.T.reshape(32, 128, C).transpose(1, 0, 2).reshape(128, 32 * C))

    wall = np.zeros((128, WALL_W), np.float32)
    wall[:, 0:512] = _pack_rows(np.ascontiguousarray(H.T.astype(np.float32)), 2)
    wall[:, 512:1536] = _pack_rows(np.ascontiguousarray(W1p.T.astype(np.float32)), 2)
    wall[:, 1536:2560] = _pack_rows(W2.T, 4)
    wall[:, 2560:2564] = hsb.astype(np.float32)

    rows = np.zeros((1, C + 2 * HID), np.float32)
    rows[0, 0:C] = r_vec.astype(np.float32)
    rows[0, C:C + HID] = w1s_row
    rows[0, C + HID:C + 2 * HID] = w1b_row

    smalls = np.zeros((128, 6), np.float32)
    smalls[:, 0:4] = b1_p.reshape(4, 128).T
    smalls[:, 4] = b2[0:128]
    smalls[:, 5] = b2[128:256]

    common = {
        "y_cm": g(y_cm_pk),
        "y_tm": g(y_tm),
        "wall": g(wall),
        "rows_mm": rows,
        "smalls": smalls,
    }
    in_maps = []
    for i in range(NCORES):
        m = dict(common)
        xs = np.ascontiguousarray(x_cm[:, i * QS:(i + 1) * QS])
        xgb = np.zeros((128, 1536), np.float32)
        xgb[:, 0:1024] = _pack_rows(xs, 2)
        xgb[:, 1024:1536] = _pack_rows(np.ascontiguousarray(G.astype(np.float32)), 2)
        m["xg"] = g(xgb)
        m["x_f32"] = _pack_rows(xs, 2)
        in_maps.append(m)
    return in_maps, dict(has_bq=has_bq, has_bo=has_bo, vb=vb)


def kernel(**inputs):
    in_maps, flags = prep_in_maps(**inputs)
    nc = _get_nc(**flags)
    res = bass_utils.run_bass_kernel_spmd(nc, in_maps, core_ids=list(range(NCORES)))
    t = np.concatenate([res.results[i]["out_sh"] for i in range(NCORES)], axis=1)
    return t.reshape(1, C, 64, 64)
